# revision 1
# baseline (speedup 1.0000x reference)
"""BSDE solver kernel for Trainium2 (8 NeuronCores, data-parallel over paths).

Math (per path):
  S_t follows GBM: S_{t+1} = S_t * u_t,  u_t = 1 + R*DT + sigma*dw_t  (autonomous)
  Y_50 = c1^50 * Y0 + sum_t c1^(49-t) * zeta_t * sigma * S_t * dw_t,  c1 = 1+R*DT
  zeta_t = sigmoid(MLP(S_t/S0, t_t)) evaluated at B*50 independent points.

So the 50-step recurrence collapses to: bulk elementwise precompute (S-hat
cumulative product, v-tilde weights), one giant batched MLP over 1.6M points
(feature-major tiles on the tensor engine, LayerNorm via weight-centering +
rsqrt Newton iteration), and a weighted reduction.

LayerNorm trick: weights are column-centered on the host so z has exactly zero
feature-mean; gamma is folded into the weights and the variance matmul uses
1/gamma^2 as the reduction vector; beta is applied as the per-partition bias of
the Gelu activation (features live on partitions in feature-major layout).
Sigmoid is computed as 0.5*(1+tanh(x/2)) so all ACT functions (Identity, Gelu,
Tanh) come from one activation table set (no table-switch stalls).
"""

import sys

sys.path.insert(0, "/opt/trn_rl_repo")

import numpy as np

import concourse.bass as bass
import concourse.bacc as bacc
import concourse.tile as tile
import concourse.mybir as mybir
import concourse.bass_utils as bass_utils

F32 = mybir.dt.float32
F32R = mybir.dt.float32r
MMDT = F32  # matmul operand dtype: fp32 = 4 cyc/row but exact; PE hides under DVE/ACT walls
I32 = mybir.dt.int32
ALU = mybir.AluOpType
ACTF = mybir.ActivationFunctionType

# Problem constants (hardcoded per spec).
B, MSTEPS, H = 32768, 50, 64
S0, R, SIGMA = 100.0, 0.05, 0.2
DT = 1.0 / MSTEPS
C1 = 1.0 + R * DT
EPS = 1e-5
NCORES = 8
NT = 400  # matmul tile free-size (divides the 50*G per-partition segment)
MAGIC = 0x5F3759DF
NR_ITERS = 2

# chunk i -> (pair, half) placement of its [64, NT] fm tile inside [128, NT]
# pair-tiles.  Layer-2 uses a swapped map so all four 64x64 matmuls can run in
# disjoint PE array quadrants.
L1MAP = lambda i: (i // 2, i % 2)
L2MAP = lambda i: (i % 2, i // 2)


def _prep_weights(ins):
    """Host-side weight preprocessing (tiny, O(H^2))."""

    def prep(W, b, g):
        Wc = (W.astype(np.float64) - W.astype(np.float64).mean(axis=1, keepdims=True))
        bc = b.astype(np.float64)
        bc = bc - bc.mean()
        return (Wc * g[None, :]).astype(np.float32), (bc * g).astype(np.float32)

    W1g, b1g = prep(ins["W1"], ins["b1"], ins["g1"])
    W2g, b2g = prep(ins["W2"], ins["b2"], ins["g2"])
    ig1 = (1.0 / ins["g1"].astype(np.float64) ** 2).astype(np.float32)
    ig2 = (1.0 / ins["g2"].astype(np.float64) ** 2).astype(np.float32)

    d = {}
    w14 = np.zeros((128, H), np.float32)
    for i in range(4):
        w14[32 * i : 32 * i + 2, :] = W1g
    d["w14"] = w14
    d["w22"] = np.concatenate([W2g, W2g], axis=0)  # [128,64]
    w3 = ins["W3"].reshape(H).astype(np.float32)
    w3p = np.zeros((H, 32, 32), np.float32)
    for dlt in range(32):
        w3p[:, dlt, dlt] = w3
    d["w3p"] = np.concatenate([w3p, w3p], axis=0).reshape(128, 32 * 32)
    d["onesc"] = np.ones((128, H), np.float32)
    igp = np.zeros((H, 2, 32, 32), np.float32)
    for dlt in range(32):
        igp[:, 0, dlt, dlt] = ig1
        igp[:, 1, dlt, dlt] = ig2
    d["igp"] = np.concatenate([igp, igp], axis=0).reshape(128, 2 * 32 * 32)
    d["b1c"] = np.tile(b1g.reshape(H, 1), (2, 1))
    d["b2c"] = np.tile(b2g.reshape(H, 1), (2, 1))
    d["be1c"] = np.tile(ins["be1"].reshape(H, 1).astype(np.float32), (2, 1))
    d["be2c"] = np.tile(ins["be2"].reshape(H, 1).astype(np.float32), (2, 1))
    d["b3h"] = np.full((128, 1), 0.5 * float(ins["b3"][0]), np.float32)
    d["y0c"] = np.full((128, 1), (C1**MSTEPS) * float(ins["Y0"][0]), np.float32)
    return d


def _afull(G):
    A = (C1 ** (MSTEPS - 1 - np.arange(MSTEPS)) * SIGMA * S0).astype(np.float32)
    return np.tile(A.reshape(1, 1, MSTEPS), (128, G, 1)).reshape(128, G * MSTEPS)


CONS_SPECS = {
    "w14": [128, H], "w22": [128, H], "w3p": [128, 32 * 32], "onesc": [128, H],
    "igp": [128, 2 * 32 * 32], "b1c": [128, 1], "b2c": [128, 1], "be1c": [128, 1],
    "be2c": [128, 1], "b3h": [128, 1], "y0c": [128, 1],
}


def build_program(G=32, wave=16, gelu=ACTF.Gelu):
    """Build the per-core Bass program. G = path-groups per partition (BC=128*G)."""
    BC = 128 * G
    SEG = G * MSTEPS  # per-partition fm segment length
    PB = SEG // NT  # blocks per partition-segment
    assert SEG % NT == 0
    NBLK = 32 * SEG // NT  # col-blocks (each spans all 4 chunk-rows)
    assert NBLK % wave == 0
    GR = NT // MSTEPS  # path-groups per mm tile (8)
    NROWS = 32 + 2 * wave  # stacked rows incl. region padding (region cg=hf)

    nc = bacc.Bacc("TRN2", target_bir_lowering=False, debug=False, num_devices=NCORES)

    dw_d = nc.dram_tensor("dw", [BC, MSTEPS], F32, kind="ExternalInput")
    tg_d = nc.dram_tensor("tg", [BC, MSTEPS], F32, kind="ExternalInput")
    cons_d = {k: nc.dram_tensor(k, s, F32, kind="ExternalInput") for k, s in CONS_SPECS.items()}
    af_d = nc.dram_tensor("afull", [128, SEG], F32, kind="ExternalInput")
    yo_d = nc.dram_tensor("yo", [BC, 1], F32, kind="ExternalOutput")
    so_d = nc.dram_tensor("so", [BC, 1], F32, kind="ExternalOutput")

    with tile.TileContext(nc) as tc:
        with (
            tc.tile_pool(name="cons", bufs=1) as cpool,
            tc.tile_pool(name="bm", bufs=1) as bmpool,
            tc.tile_pool(name="x4", bufs=3) as x4pool,
            tc.tile_pool(name="zs", bufs=40) as zspool,
            tc.tile_pool(name="zsq", bufs=3) as zsqpool,
            tc.tile_pool(name="h", bufs=6) as hpool,
            tc.tile_pool(name="nr", bufs=2) as nrpool,
            tc.tile_pool(name="nri", bufs=2) as nripool,
            tc.tile_pool(name="rsl", bufs=2) as rslpool,
            tc.tile_pool(name="scr", bufs=1) as scrpool,
            tc.tile_pool(name="zp", bufs=3, space="PSUM") as zppool,
            tc.tile_pool(name="ssp", bufs=2, space="PSUM") as sspool,
            tc.tile_pool(name="rbp", bufs=3, space="PSUM") as rbpool,
        ):
            # ---- load constants ----
            MMCONS = ("w14", "w22", "onesc", "igp", "w3p")
            cons, consr = {}, {}
            for k, s in CONS_SPECS.items():
                if k in MMCONS:
                    continue
                t = cpool.tile(s, F32, tag=k)
                nc.sync.dma_start(t[:], cons_d[k].ap())
                cons[k] = t
            # f32r-rounded copies of matmul operand consts (staged via scratch)
            for k in MMCONS:
                stg = scrpool.tile([128, 2 * 32 * 32], F32, tag="scr")
                s = CONS_SPECS[k]
                nc.sync.dma_start(stg[:, : s[1]], cons_d[k].ap())
                tr = cpool.tile(s, MMDT, tag=k + "r")
                nc.scalar.activation(tr[:], stg[:, : s[1]], ACTF.Identity)
                consr[k] = tr
            af = cpool.tile([128, SEG], F32, tag="afull")
            nc.sync.dma_start(af[:], af_d.ap())

            # ---- phase A: batch-major precompute ----
            dwb = bmpool.tile([128, SEG], F32, tag="dwb")
            nc.sync.dma_start(dwb[:], dw_d.ap().rearrange("(p g) t -> p (g t)", p=128))
            u = bmpool.tile([128, SEG], F32, tag="u")
            nc.vector.tensor_scalar(u[:], dwb[:], SIGMA, 1.0 + R * DT, ALU.mult, ALU.add)
            sh = bmpool.tile([128, SEG], F32, tag="sh")
            nc.vector.memset(sh[:], 1.0)
            sh3 = sh[:].rearrange("p (g t) -> p g t", t=MSTEPS)
            u3 = u[:].rearrange("p (g t) -> p g t", t=MSTEPS)
            for t in range(1, MSTEPS):
                nc.vector.tensor_tensor(sh3[:, :, t], sh3[:, :, t - 1], u3[:, :, t - 1], ALU.mult)
            vt = bmpool.tile([128, SEG], F32, tag="vt")
            nc.vector.tensor_tensor(vt[:], dwb[:], af[:], ALU.mult)
            nc.vector.tensor_tensor(vt[:], vt[:], sh[:], ALU.mult)
            sout = bmpool.tile([128, G], F32, tag="sout")
            nc.vector.scalar_tensor_tensor(
                sout[:], sh3[:, :, MSTEPS - 1], S0, u3[:, :, MSTEPS - 1], ALU.mult, ALU.mult
            )
            nc.sync.dma_start(so_d.ap().rearrange("(p g) o -> p (g o)", p=128), sout[:])

            zb = bmpool.tile([128, SEG], F32, tag="zb")
            # f32r-rounded copies of Shat and t for MLP inputs
            shr = bmpool.tile([128, SEG], MMDT, tag="shr")
            nc.scalar.activation(shr[:], sh[:], ACTF.Identity)
            tgb = scrpool.tile([128, 2 * 32 * 32], F32, tag="scr")
            nc.sync.dma_start(tgb[:, :SEG], tg_d.ap().rearrange("(p g) t -> p (g t)", p=128))
            tgr = bmpool.tile([128, SEG], MMDT, tag="tgr")
            nc.scalar.activation(tgr[:], tgb[:, :SEG], ACTF.Identity)

            def src_loc(kb, i):
                p = 32 * i + kb // PB
                roff = (kb % PB) * NT
                return p, roff

            def nr_rsqrt(sstk):
                """In-place-ish Newton rsqrt of mean(sstk)/H + EPS over NROWS rows.
                Returns rstd tile."""
                vh = nrpool.tile([128, NT], F32, tag="vh")
                nc.vector.tensor_scalar(vh[:NROWS, :], sstk[:NROWS, :], 1.0 / H, EPS, ALU.mult, ALU.add)
                sh1 = nripool.tile([128, NT], I32, tag="sh1")
                nc.vector.tensor_scalar(
                    sh1[:NROWS, :], vh[:NROWS, :].bitcast(I32), 1, None, ALU.logical_shift_right
                )
                y = nrpool.tile([128, NT], F32, tag="ynr")
                nc.vector.tensor_scalar(
                    y[:NROWS, :].bitcast(I32), sh1[:NROWS, :], -1, MAGIC, ALU.mult, ALU.add
                )
                ta = nrpool.tile([128, NT], F32, tag="ta")
                yr = nrpool.tile([128, NT], MMDT, tag="yr")
                for it in range(NR_ITERS):
                    nc.vector.tensor_tensor(ta[:NROWS, :], y[:NROWS, :], y[:NROWS, :], ALU.mult)
                    nc.vector.tensor_tensor(ta[:NROWS, :], ta[:NROWS, :], vh[:NROWS, :], ALU.mult)
                    nc.vector.tensor_scalar(ta[:NROWS, :], ta[:NROWS, :], -0.5, 1.5, ALU.mult, ALU.add)
                    dst = yr if it == NR_ITERS - 1 else y
                    nc.vector.tensor_tensor(dst[:NROWS, :], y[:NROWS, :], ta[:NROWS, :], ALU.mult)
                return yr

            def rows4(ap):
                """AP over partitions {0,32,64,96} of a [128, NT] tile -> [4, NT]."""
                return ap[:].rearrange("(a b) n -> a b n", b=32)[:, 0, :]

            # ---- phase B: software-pipelined waves ----
            # Stacked-row trick: SS / z3 matmuls use zero-padded [64,32]
            # stationary slabs so each block-chunk's row lands at a distinct
            # partition of one shared PSUM bank (accumulating into disjoint
            # rows).  NR rsqrt then runs on 4*wave rows at once.
            igp4 = consr["igp"][:].rearrange("p (l d m) -> p l d m", l=2, d=32)
            w3p4 = consr["w3p"][:].rearrange("p (d m) -> p d m", d=32)

            # HW constraint: a PSUM accumulation group must keep one
            # tile_position.  So region cg=hf collects only rows whose rhs
            # lives at partition base 64*hf; row-within-region = 2*bi + j.
            def stacked_mm(stk, bi, j, hf, lhs_slab, rhs):
                dlt = 2 * bi + j
                nc.tensor.matmul(
                    stk[32 * hf : 32 * hf + 32, :],
                    lhs_slab(dlt, hf),
                    rhs,
                    start=(dlt == 0), stop=(dlt == 2 * wave - 1),
                    tile_position=(64 * hf, 32 * hf),
                    skip_group_check=True,
                )

            def stkrow(bi, i, lmap):
                pr, hf = lmap(i)
                j = (i // 2) if lmap is L1MAP else (i % 2)
                return 32 * hf + 2 * bi + j

            for wstart in range(0, NBLK, wave):
                blocks = range(wstart, wstart + wave)
                # -- loop1: X4 build, L1 matmul, bias/extract, square, SS1 --
                zs1 = []
                sstk1 = sspool.tile([128, NT], F32, tag="stk")
                for bi, kb in enumerate(blocks):
                    x4 = x4pool.tile([128, NT], MMDT, tag="x4")
                    for i in range(4):
                        p, roff = src_loc(kb, i)
                        nc.sync.dma_start(x4[32 * i : 32 * i + 1, :], shr[p : p + 1, roff : roff + NT])
                        nc.sync.dma_start(x4[32 * i + 1 : 32 * i + 2, :], tgr[p : p + 1, roff : roff + NT])
                    zpa = zppool.tile([128, NT], F32, tag="zp")
                    zpb = zppool.tile([128, NT], F32, tag="zp")
                    zpair = (zpa, zpb)
                    for i in range(4):
                        pr, hf = L1MAP(i)
                        nc.tensor.matmul(
                            zpair[pr][64 * hf : 64 * hf + 64, :],
                            consr["w14"][32 * i : 32 * i + 2, :],
                            x4[32 * i : 32 * i + 2, :],
                            start=True, stop=True, tile_position=(32 * i, 64 * hf),
                        )
                    zs_a = zspool.tile([128, NT], F32, tag="zs")
                    zs_b = zspool.tile([128, NT], F32, tag="zs")
                    nc.scalar.activation(zs_a[:], zpa[:], ACTF.Identity, bias=cons["b1c"][:])
                    nc.scalar.activation(zs_b[:], zpb[:], ACTF.Identity, bias=cons["b1c"][:])
                    q_a = zsqpool.tile([128, NT], MMDT, tag="zsq")
                    q_b = zsqpool.tile([128, NT], MMDT, tag="zsq")
                    nc.vector.tensor_tensor(q_a[:], zs_a[:], zs_a[:], ALU.mult)
                    nc.vector.tensor_tensor(q_b[:], zs_b[:], zs_b[:], ALU.mult)
                    qp = (q_a, q_b)
                    for i in range(4):
                        pr, hf = L1MAP(i)
                        stacked_mm(
                            sstk1, bi, i // 2, hf,
                            lambda dlt, hf_: igp4[64 * hf_ : 64 * hf_ + 64, 0, dlt, :],
                            qp[pr][64 * hf : 64 * hf + 64, :],
                        )
                    zs1.append((zs_a, zs_b))
                rstd1 = nr_rsqrt(sstk1)

                # -- loop2: LN1 apply + gelu -> h1; L2 matmul; square; SS2 --
                zs2 = []
                sstk2 = sspool.tile([128, NT], F32, tag="stk")
                for bi, kb in enumerate(blocks):
                    rsl = rslpool.tile([128, NT], MMDT, tag="rsl")
                    for i in range(4):
                        r = stkrow(bi, i, L1MAP)
                        nc.sync.dma_start(rsl[32 * i : 32 * i + 1, :], rstd1[r : r + 1, :])
                    rba = rbpool.tile([128, NT], F32, tag="rb")
                    rbb = rbpool.tile([128, NT], F32, tag="rb")
                    rpair = (rba, rbb)
                    for i in range(4):
                        pr, hf = L1MAP(i)
                        nc.tensor.matmul(
                            rpair[pr][64 * hf : 64 * hf + 64, :],
                            consr["onesc"][32 * i : 32 * i + 1, :],
                            rsl[32 * i : 32 * i + 1, :],
                            start=True, stop=True, tile_position=(32 * i, 64 * hf),
                        )
                    zs_a, zs_b = zs1[bi]
                    h_a = hpool.tile([128, NT], MMDT, tag="h")
                    h_b = hpool.tile([128, NT], MMDT, tag="h")
                    for h_, zs_, rb_ in ((h_a, zs_a, rba), (h_b, zs_b, rbb)):
                        nc.vector.tensor_tensor(zs_[:], zs_[:], rb_[:], ALU.mult)
                        nc.scalar.activation(h_[:], zs_[:], gelu, bias=cons["be1c"][:])
                    hpair = (h_a, h_b)
                    zpa = zppool.tile([128, NT], F32, tag="zp")
                    zpb = zppool.tile([128, NT], F32, tag="zp")
                    zpair = (zpa, zpb)
                    for i in range(4):
                        spr, shf = L1MAP(i)  # where h1 of chunk i lives
                        pr, hf = L2MAP(i)  # where z2 of chunk i goes
                        nc.tensor.matmul(
                            zpair[pr][64 * hf : 64 * hf + 64, :],
                            consr["w22"][64 * shf : 64 * shf + 64, :],
                            hpair[spr][64 * shf : 64 * shf + 64, :],
                            start=True, stop=True, tile_position=(64 * shf, 64 * hf),
                        )
                    zs_a2 = zspool.tile([128, NT], F32, tag="zs")
                    zs_b2 = zspool.tile([128, NT], F32, tag="zs")
                    nc.scalar.activation(zs_a2[:], zpa[:], ACTF.Identity, bias=cons["b2c"][:])
                    nc.scalar.activation(zs_b2[:], zpb[:], ACTF.Identity, bias=cons["b2c"][:])
                    q_a = zsqpool.tile([128, NT], MMDT, tag="zsq")
                    q_b = zsqpool.tile([128, NT], MMDT, tag="zsq")
                    nc.vector.tensor_tensor(q_a[:], zs_a2[:], zs_a2[:], ALU.mult)
                    nc.vector.tensor_tensor(q_b[:], zs_b2[:], zs_b2[:], ALU.mult)
                    qp = (q_a, q_b)
                    for i in range(4):
                        pr, hf = L2MAP(i)
                        stacked_mm(
                            sstk2, bi, i % 2, hf,
                            lambda dlt, hf_: igp4[64 * hf_ : 64 * hf_ + 64, 1, dlt, :],
                            qp[pr][64 * hf : 64 * hf + 64, :],
                        )
                    zs2.append((zs_a2, zs_b2))
                rstd2 = nr_rsqrt(sstk2)

                # -- loop3: LN2 apply + gelu -> h2; L3; zeta back to bm --
                zstk = sspool.tile([128, NT], F32, tag="stk")
                for bi, kb in enumerate(blocks):
                    rsl = rslpool.tile([128, NT], MMDT, tag="rsl")
                    for i in range(4):
                        r = stkrow(bi, i, L2MAP)
                        nc.sync.dma_start(rsl[32 * i : 32 * i + 1, :], rstd2[r : r + 1, :])
                    rba = rbpool.tile([128, NT], F32, tag="rb")
                    rbb = rbpool.tile([128, NT], F32, tag="rb")
                    rpair = (rba, rbb)
                    for i in range(4):
                        pr, hf = L2MAP(i)
                        nc.tensor.matmul(
                            rpair[pr][64 * hf : 64 * hf + 64, :],
                            consr["onesc"][32 * i : 32 * i + 1, :],
                            rsl[32 * i : 32 * i + 1, :],
                            start=True, stop=True, tile_position=(32 * i, 64 * hf),
                        )
                    zs_a2, zs_b2 = zs2[bi]
                    h_a = hpool.tile([128, NT], MMDT, tag="h")
                    h_b = hpool.tile([128, NT], MMDT, tag="h")
                    for h_, zs_, rb_ in ((h_a, zs_a2, rba), (h_b, zs_b2, rbb)):
                        nc.vector.tensor_tensor(zs_[:], zs_[:], rb_[:], ALU.mult)
                        nc.scalar.activation(h_[:], zs_[:], gelu, bias=cons["be2c"][:])
                    hpair = (h_a, h_b)
                    for i in range(4):
                        pr, hf = L2MAP(i)
                        stacked_mm(
                            zstk, bi, i % 2, hf,
                            lambda dlt, hf_: w3p4[64 * hf_ : 64 * hf_ + 64, dlt, :],
                            hpair[pr][64 * hf : 64 * hf + 64, :],
                        )
                # extract zeta rows: one ACT copy psum->sbuf, then row DMAs
                zsc = rslpool.tile([128, NT], F32, tag="zsc")
                nc.scalar.activation(zsc[:NROWS, :], zstk[:NROWS, :], ACTF.Identity)
                for bi, kb in enumerate(blocks):
                    for i in range(4):
                        p, roff = src_loc(kb, i)
                        r = stkrow(bi, i, L2MAP)
                        nc.sync.dma_start(
                            zb[p : p + 1, roff : roff + NT],
                            zsc[r : r + 1, :],
                        )

            # ---- phase C: zeta -> Y ----
            tbm = bmpool.tile([128, SEG], F32, tag="dwb")
            nc.scalar.activation(tbm[:], zb[:], ACTF.Tanh, bias=cons["b3h"][:], scale=0.5)
            nc.vector.scalar_tensor_tensor(tbm[:], tbm[:], 1.0, vt[:], ALU.add, ALU.mult)
            ps = bmpool.tile([128, G], F32, tag="ps")
            nc.vector.tensor_reduce(
                ps[:], tbm[:].rearrange("p (g t) -> p g t", t=MSTEPS), mybir.AxisListType.X, ALU.add
            )
            yout = bmpool.tile([128, G], F32, tag="yout")
            nc.vector.tensor_scalar(yout[:], ps[:], 0.5, cons["y0c"][:], ALU.mult, ALU.add)
            nc.sync.dma_start(yo_d.ap().rearrange("(p g) o -> p (g o)", p=128), yout[:])

    nc.compile()
    return nc


_CACHE = {}


def _get_program(G=32, wave=16):
    key = (G, wave)
    if key not in _CACHE:
        _CACHE[key] = build_program(G, wave)
    return _CACHE[key]


def make_in_maps(inputs, G=32):
    BC = 128 * G
    cons = _prep_weights(inputs)
    cons["afull"] = _afull(G)
    dw = np.ascontiguousarray(np.asarray(inputs["dw"], np.float32)[: NCORES * BC])
    tg = np.ascontiguousarray(np.asarray(inputs["t_grid"], np.float32)[: NCORES * BC])
    maps = []
    for c in range(NCORES):
        m = {"dw": dw[c * BC : (c + 1) * BC], "tg": tg[c * BC : (c + 1) * BC]}
        m.update(cons)
        maps.append(m)
    return maps


def kernel(**inputs):
    nc = _get_program()
    in_maps = make_in_maps(inputs)
    res = bass_utils.run_bass_kernel_spmd(nc, in_maps, core_ids=list(range(NCORES)))
    Y = np.concatenate([res.results[c]["yo"] for c in range(NCORES)], axis=0)
    S = np.concatenate([res.results[c]["so"] for c in range(NCORES)], axis=0)
    return Y.reshape(B, 1).astype(np.float32), S.reshape(B, 1).astype(np.float32)



# revision 40
# speedup vs baseline: 16.5247x; 16.5247x over previous
"""BSDE solver kernel for Trainium2 (8 NeuronCores, data-parallel over paths).

Math (per path):
  S_t follows GBM: S_{t+1} = S_t * u_t,  u_t = 1 + R*DT + sigma*dw_t  (autonomous)
  Y_50 = c1^50 * Y0 + sum_t c1^(49-t) * zeta_t * sigma * S_t * dw_t,  c1 = 1+R*DT
  zeta_t = sigmoid(MLP(S_t/S0, t_t)) evaluated at B*50 independent points.

So the 50-step recurrence collapses to: bulk elementwise precompute (S-hat
cumulative product, v-tilde weights), one giant batched MLP over 1.6M points
(feature-major on the tensor engine, LayerNorm via weight-centering + Newton
rsqrt), and a weighted reduction.

Layout: a wave = 16 blocks x 400 points.  The MLP inputs for a whole wave live
in ONE [128, 400] tile (row = 32*chunk + 16*feature + block), gathered from
the batch-major S-hat/t tiles with eight [16,400] contiguous-row DMAs (DMA
cost here is ~0.39ns per row-BYTE, row count nearly free — so shuffles must be
many-short-rows, never one long row).  Each per-block matmul is a single
full-K instruction whose stationary is a 128-wide column slice of a
16-variant constant (variant = block): L1 input slabs, rstd-broadcast
selectors (reading the Newton-rsqrt output tile directly as the moving
operand — no rstd staging copies at all), and sliding one-hot variance/L3
slabs that accumulate a whole wave into one [64,400] PSUM stack
(row = 32*pair + 16*j + block).  Zeta returns to batch-major as bf16 with
four [4,1600]<-[16,400] scatters per wave.

LayerNorm trick: weights are column-centered on the host so z has exactly
zero feature-mean; gamma folds into the weights, the variance matmul uses
1/gamma^2, beta rides the Gelu bias.  Sigmoid = 0.5*(1+tanh(x/2)) keeps all
ACT functions in one table set.  Datapath is bf16 (1 cyc/row matmuls; rel-err
budget 2e-2, bf16 lands ~6e-4).  GPSIMD/Pool cannot access PSUM on real HW,
so PSUM-side work (evictions, LN applies) is split between ACT and DVE —
z lives in 2-bank [128,1024] PSUM tiles whose two 400-col halves are
processed by single strided-AP ops — while Pool takes the SBUF-only squares
and the Newton-rsqrt tail.  Emission is stage-skewed across blocks and
loop3(w) interleaves with loop1(w+1) to hide the rsqrt barriers.
"""

import sys

sys.path.insert(0, "/opt/trn_rl_repo")

import numpy as np

import concourse.bass as bass
import concourse.bacc as bacc
import concourse.tile as tile
import concourse.mybir as mybir
import concourse.bass_utils as bass_utils

F32 = mybir.dt.float32
MMDT = mybir.dt.bfloat16  # MLP datapath dtype
I32 = mybir.dt.int32
ALU = mybir.AluOpType
ACTF = mybir.ActivationFunctionType

# Problem constants (hardcoded per spec).
B, MSTEPS, H = 32768, 50, 64
S0, R, SIGMA = 100.0, 0.05, 0.2
DT = 1.0 / MSTEPS
C1 = 1.0 + R * DT
EPS = 1e-5
NCORES = 8
NT = 400  # block free-size (divides the 50*G per-partition segment)
WAVE = 32  # blocks per wave: stack rows 64*pair+32*j+bi fill all 128 partitions
MAGIC = 0x5F3759DF
NR_ITERS = 1  # magic-guess + 1 Newton step: rstd rel err ~1.7e-3, under bf16 noise
SLC = 95  # slab marker column: slab slice for (pair,bi) starts at SLC-64*pair-bi


def _prep_weights(ins):
    """Host-side weight preprocessing (tiny, O(H^2))."""

    def prep(W, b, g):
        Wc = (W.astype(np.float64) - W.astype(np.float64).mean(axis=1, keepdims=True))
        bc = b.astype(np.float64)
        bc = bc - bc.mean()
        return (Wc * g[None, :]).astype(np.float32), (bc * g).astype(np.float32)

    W1g, b1g = prep(ins["W1"], ins["b1"], ins["g1"])
    W2g, b2g = prep(ins["W2"], ins["b2"], ins["g2"])
    ig1 = (1.0 / ins["g1"].astype(np.float64) ** 2).astype(np.float32)
    ig2 = (1.0 / ins["g2"].astype(np.float64) ** 2).astype(np.float32)

    d = {}
    # L1 stationary, 16 variants (one per block in a wave): within each
    # 64-row pair half, moving row 32*c2 + 16*q + bi holds input feature q of
    # chunk c2 for block bi -> maps to W1g[q] at output cols [64*c2, 64*c2+64)
    w14v = np.zeros((128, 16 * 128), np.float32)
    for ph in (0, 64):
        for bi in range(16):
            for c2 in (0, 1):
                for q in (0, 1):
                    w14v[ph + 32 * c2 + 16 * q + bi,
                         128 * bi + 64 * c2 : 128 * bi + 64 * c2 + 64] = W1g[q]
    d["w14v"] = w14v
    w22bd = np.zeros((128, 128), np.float32)
    w22bd[0:64, 0:64] = W2g
    w22bd[64:128, 64:128] = W2g
    d["w22bd"] = w22bd
    # rstd-broadcast selector, 32 variants: out rows [0,64) take stack row
    # 64p+bi (j=0), rows [64,128) take 64p+32+bi (j=1); K=64 moving = the
    # Newton-rsqrt output rows [64p, 64p+64) read in place.
    selc = np.zeros((128, 32 * 128), np.float32)
    for ph in (0, 64):
        for bi in range(32):
            selc[ph + bi, 128 * bi : 128 * bi + 64] = 1.0
            selc[ph + 32 + bi, 128 * bi + 64 : 128 * bi + 128] = 1.0
    d["selc"] = selc
    w3 = ins["W3"].reshape(H).astype(np.float32)

    def slab(vec):
        s = np.zeros((128, 224), np.float32)
        s[0:64, SLC] = vec
        s[64:128, SLC + 32] = vec
        return s

    d["igsl1"] = slab(ig1)
    d["igsl2"] = slab(ig2)
    d["w3sl"] = slab(w3)
    d["b1c"] = np.tile(b1g.reshape(H, 1), (2, 1))
    d["b2c"] = np.tile(b2g.reshape(H, 1), (2, 1))
    d["be1c"] = np.tile(ins["be1"].reshape(H, 1).astype(np.float32), (2, 1))
    d["be2c"] = np.tile(ins["be2"].reshape(H, 1).astype(np.float32), (2, 1))
    d["b3h"] = np.full((128, 1), 0.5 * float(ins["b3"][0]), np.float32)
    d["y0c"] = np.full((128, 1), (C1**MSTEPS) * float(ins["Y0"][0]), np.float32)
    return d


def _afull(G):
    A = (C1 ** (MSTEPS - 1 - np.arange(MSTEPS)) * SIGMA * S0).astype(np.float32)
    return np.tile(A.reshape(1, 1, MSTEPS), (128, G, 1)).reshape(128, G * MSTEPS)


CONS_SPECS = {
    "w14v": [128, 2048], "w22bd": [128, 128], "selc": [128, 4096],
    "igsl1": [128, 224], "igsl2": [128, 224], "w3sl": [128, 224],
    "b1c": [128, 1], "b2c": [128, 1], "be1c": [128, 1],
    "be2c": [128, 1], "b3h": [128, 1], "y0c": [128, 1],
}
MMCONS = ("w14v", "w22bd", "selc", "igsl1", "igsl2", "w3sl")


def build_program(G=32, gelu=ACTF.Gelu):
    """Build the per-core Bass program. G = path-groups per partition (BC=128*G)."""
    BC = 128 * G
    SEG = G * MSTEPS  # per-partition fm segment length
    PB = SEG // NT  # blocks per partition-segment
    assert SEG % NT == 0
    NBLK = 32 * SEG // NT  # col-blocks (each spans all 4 chunk-rows)
    assert NBLK % WAVE == 0
    HWV = WAVE // PB  # partitions-worth of blocks per wave
    assert WAVE == HWV * PB

    nc = bacc.Bacc("TRN2", target_bir_lowering=False, debug=False, num_devices=NCORES)

    dw_d = nc.dram_tensor("dw", [BC, MSTEPS], F32, kind="ExternalInput")
    tg_d = nc.dram_tensor("tg", [BC, MSTEPS], F32, kind="ExternalInput")
    cons_d = {k: nc.dram_tensor(k, s, F32, kind="ExternalInput") for k, s in CONS_SPECS.items()}
    af_d = nc.dram_tensor("afull", [128, SEG], F32, kind="ExternalInput")
    yo_d = nc.dram_tensor("yo", [BC, 1], F32, kind="ExternalOutput")
    so_d = nc.dram_tensor("so", [BC, 1], F32, kind="ExternalOutput")

    with tile.TileContext(nc) as tc:
        with (
            tc.tile_pool(name="cons", bufs=1) as cpool,
            tc.tile_pool(name="bm", bufs=1) as bmpool,
            tc.tile_pool(name="x4", bufs=4) as x4pool,
            tc.tile_pool(name="zs", bufs=66) as zspool,
            tc.tile_pool(name="q", bufs=3) as qpool,
            tc.tile_pool(name="h", bufs=4) as hpool,
            tc.tile_pool(name="nr", bufs=2) as nrpool,
            tc.tile_pool(name="nri", bufs=2) as nripool,
            tc.tile_pool(name="zc", bufs=2) as zcpool,
            tc.tile_pool(name="scr", bufs=1) as scrpool,
            tc.tile_pool(name="zp", bufs=3, space="PSUM") as zppool,
            tc.tile_pool(name="ssp", bufs=2, space="PSUM") as sspool,
        ):
            # ---- load constants ----
            cons, consr = {}, {}
            for k, s in CONS_SPECS.items():
                if k in MMCONS:
                    continue
                t = cpool.tile(s, F32, tag=k)
                nc.sync.dma_start(t[:], cons_d[k].ap())
                cons[k] = t
            # bf16 copies of matmul operand consts (staged via scratch)
            for k in MMCONS:
                s = CONS_SPECS[k]
                tr = cpool.tile(s, MMDT, tag=k + "r")
                for c0 in range(0, s[1], 2048):
                    cw = min(2048, s[1] - c0)
                    stg = scrpool.tile([128, 2048], F32, tag="scr")
                    nc.sync.dma_start(stg[: s[0], :cw], cons_d[k].ap()[:, c0 : c0 + cw])
                    nc.scalar.activation(tr[:, c0 : c0 + cw], stg[: s[0], :cw], ACTF.Identity)
                consr[k] = tr
            af = cpool.tile([128, SEG], F32, tag="afull")
            nc.sync.dma_start(af[:], af_d.ap())

            # ---- phase A: batch-major precompute ----
            dwb = bmpool.tile([128, SEG], F32, tag="dwb")
            nc.sync.dma_start(dwb[:], dw_d.ap().rearrange("(p g) t -> p (g t)", p=128))
            u = bmpool.tile([128, SEG], F32, tag="u")
            nc.gpsimd.tensor_scalar(u[:], dwb[:], SIGMA, 1.0 + R * DT, ALU.mult, ALU.add)
            sh = bmpool.tile([128, SEG], F32, tag="sh")
            nc.vector.memset(sh[:], 1.0)
            sh3 = sh[:].rearrange("p (g t) -> p g t", t=MSTEPS)
            u3 = u[:].rearrange("p (g t) -> p g t", t=MSTEPS)
            for t in range(1, MSTEPS):
                nc.vector.tensor_tensor(sh3[:, :, t], sh3[:, :, t - 1], u3[:, :, t - 1], ALU.mult)
            vt = bmpool.tile([128, SEG], F32, tag="vt")
            nc.gpsimd.tensor_tensor(vt[:], dwb[:], af[:], ALU.mult)
            nc.gpsimd.tensor_tensor(vt[:], vt[:], sh[:], ALU.mult)
            sout = bmpool.tile([128, G], F32, tag="sout")
            nc.vector.scalar_tensor_tensor(
                sout[:], sh3[:, :, MSTEPS - 1], S0, u3[:, :, MSTEPS - 1], ALU.mult, ALU.mult
            )
            nc.sync.dma_start(so_d.ap().rearrange("(p g) o -> p (g o)", p=128), sout[:])

            zb = bmpool.tile([128, SEG], MMDT, tag="zb")
            # bf16 copies of Shat and t for MLP inputs
            shr = bmpool.tile([128, SEG], MMDT, tag="shr")
            nc.scalar.activation(shr[:], sh[:], ACTF.Identity)
            tgb = scrpool.tile([128, 2048], F32, tag="scr")
            nc.sync.dma_start(tgb[:, :SEG], tg_d.ap().rearrange("(p g) t -> p (g t)", p=128))
            tgr = bmpool.tile([128, SEG], MMDT, tag="tgr")
            nc.scalar.activation(tgr[:], tgb[:, :SEG], ACTF.Identity)

            def nr_rsqrt(sstk, eng):
                """Newton rsqrt of mean(sstk)/H over 64 stack rows (EPS
                dropped: variance >> 1e-5 here).  rsqrt(s/64) = 8*rsqrt(s):
                Newton runs on s directly; the final step's constants fold
                in the *8.  PSUM reads go to DVE (shift) and ACT (staging
                copy, in parallel); the SBUF tail runs on `eng` (Pool)."""
                NR = 4 * WAVE
                sh1 = nripool.tile([128, NT], I32, tag="sh1")
                nc.vector.tensor_scalar(
                    sh1[:NR, :], sstk[:NR, :].bitcast(I32), 1, None, ALU.logical_shift_right
                )
                vh = nrpool.tile([128, NT], F32, tag="vh")
                nc.scalar.activation(vh[:NR, :], sstk[:NR, :], ACTF.Identity)
                y = nrpool.tile([128, NT], F32, tag="ynr")
                eng.tensor_scalar(
                    y[:NR, :].bitcast(I32), sh1[:NR, :], -1, MAGIC, ALU.mult, ALU.add
                )
                ta = nrpool.tile([128, NT], F32, tag="ta")
                yr = nrpool.tile([128, NT], MMDT, tag="yr")
                eng.tensor_tensor(ta[:NR, :], y[:NR, :], y[:NR, :], ALU.mult)
                eng.tensor_tensor(ta[:NR, :], ta[:NR, :], vh[:NR, :], ALU.mult)
                eng.tensor_scalar(ta[:NR, :], ta[:NR, :], -4.0, 12.0, ALU.mult, ALU.add)
                eng.tensor_tensor(yr[:NR, :], y[:NR, :], ta[:NR, :], ALU.mult)
                return yr

            # ---- phase B: waves of 16 blocks ----
            # Wave-input tile x4v [128, NT]: row = 32*chunk + 16*q + bi.
            # Stack row for (pair p, j, block bi) = 32p + 16j + bi.
            shv = shr[:].rearrange("p (l c) -> p l c", l=PB)
            tgv = tgr[:].rearrange("p (l c) -> p l c", l=PB)

            def slab_mm(stk, bi, p, slabc, rhs, first, last):
                nc.tensor.matmul(
                    stk[0:128, :],
                    slabc[:, SLC - 64 * p - bi : SLC - 64 * p - bi + 128],
                    rhs,
                    start=first, stop=last,
                    tile_position=(0, 0),
                    skip_group_check=True,
                )

            NW = NBLK // WAVE

            def gather(w):
                halves = []
                for hh in (0, 1):
                    x4v = x4pool.tile([128, NT], MMDT, tag="x4v", name=f"x4v{w}_{hh}")
                    for i in range(4):
                        p0 = 32 * i + HWV * w + 4 * hh
                        nc.sync.dma_start(x4v[32 * i : 32 * i + 16, :], shv[p0 : p0 + 4])
                        nc.sync.dma_start(x4v[32 * i + 16 : 32 * i + 32, :], tgv[p0 : p0 + 4])
                    halves.append(x4v)
                return halves

            # Stage-skewed emission: per-engine queues are in-order, so each
            # k-iteration interleaves independent stages of consecutive
            # blocks (block k's input matmul, block k-1's elementwise, block
            # k-2's stack matmul).  Additionally loop3 of wave w is emitted
            # interleaved with loop1 of wave w+1 (complementary engine
            # profiles, and it hides both Newton-rsqrt barriers).
            def pv(t):
                # [128, 2, NT] strided view of a 2-bank psum tile: the two
                # 400-col halves at bank offsets 0 and 512
                return t[:].rearrange("p (u c) -> p u c", u=2)[:, :, :NT]

            def sv2(t):
                return t[:].rearrange("p (u c) -> p u c", u=2)

            def a_mm(st, bi):
                zp1 = zppool.tile([128, 1024], F32, tag="zp", name=f"zp1_{bi}")
                x4h = st["x4v"][bi // 16]
                bl = bi % 16
                for p in (0, 1):
                    nc.tensor.matmul(
                        zp1[:, 512 * p : 512 * p + NT],
                        consr["w14v"][64 * p : 64 * p + 64, 128 * bl : 128 * bl + 128],
                        x4h[64 * p : 64 * p + 64, :],
                        start=True, stop=True, tile_position=(64 * p, 0),
                    )
                st["zp1"][bi] = zp1

            def a_ev(st, bi):
                zp1 = st["zp1"][bi]
                zs = zspool.tile([128, 2 * NT], MMDT, tag="zs", name=f"zs1_{bi}")
                nc.scalar.activation(zs[:, :NT], zp1[:, :NT], ACTF.Identity, bias=cons["b1c"][:])
                nc.vector.tensor_scalar(zs[:, NT:], zp1[:, 512 : 512 + NT], cons["b1c"][:], None, ALU.add)
                st["zs1"][bi] = zs

            def a_q(st, bi):
                q = qpool.tile([128, 2 * NT], MMDT, tag="q", name=f"q1_{bi}")
                nc.gpsimd.tensor_tensor(q[:], st["zs1"][bi][:], st["zs1"][bi][:], ALU.mult)
                st["q1"][bi] = q

            def a_ss(st, bi):
                for p in (0, 1):
                    slab_mm(st["sstk1"], bi, p, consr["igsl1"],
                            st["q1"][bi][:][:, p * NT : p * NT + NT],
                            first=(bi == 0 and p == 0), last=(bi == WAVE - 1 and p == 1))

            def b_rb(st, bi):
                rb1 = zppool.tile([128, 1024], F32, tag="zp", name=f"rb1_{bi}")
                for p in (0, 1):
                    nc.tensor.matmul(
                        rb1[:, 512 * p : 512 * p + NT],
                        consr["selc"][64 * p : 64 * p + 64, 128 * bi : 128 * bi + 128],
                        st["rstd1"][64 * p : 64 * p + 64, :],
                        start=True, stop=True, tile_position=(64 * p, 0),
                    )
                st["rb1"][bi] = rb1

            def b_ew(st, bi):
                rb1 = st["rb1"][bi]
                zs = st["zs1"][bi]
                nc.vector.tensor_tensor(sv2(zs), sv2(zs), pv(rb1), ALU.mult)
                h1 = hpool.tile([128, 2 * NT], MMDT, tag="h", name=f"h1_{bi}")
                nc.scalar.activation(h1[:], zs[:], gelu, bias=cons["be1c"][:])
                zp2 = zppool.tile([128, 1024], F32, tag="zp", name=f"zp2_{bi}")
                for p in (0, 1):
                    nc.tensor.matmul(
                        zp2[:, 512 * p : 512 * p + NT],
                        consr["w22bd"][:, :],
                        h1[:, p * NT : p * NT + NT],
                        start=True, stop=True, tile_position=(0, 0),
                    )
                st["zp2"][bi] = zp2

            def b_ev(st, bi):
                zp2 = st["zp2"][bi]
                zs_2 = zspool.tile([128, 2 * NT], MMDT, tag="zs", name=f"zs2_{bi}")
                nc.vector.tensor_scalar(zs_2[:, :NT], zp2[:, :NT], cons["b2c"][:], None, ALU.add)
                nc.scalar.activation(zs_2[:, NT:], zp2[:, 512 : 512 + NT], ACTF.Identity, bias=cons["b2c"][:])
                q = qpool.tile([128, 2 * NT], MMDT, tag="q", name=f"q2_{bi}")
                nc.gpsimd.tensor_tensor(q[:], zs_2[:], zs_2[:], ALU.mult)
                st["zs2"][bi], st["q2"][bi] = zs_2, q

            def b_ss(st, bi):
                for p in (0, 1):
                    slab_mm(st["sstk2"], bi, p, consr["igsl2"],
                            st["q2"][bi][:][:, p * NT : p * NT + NT],
                            first=(bi == 0 and p == 0), last=(bi == WAVE - 1 and p == 1))

            def c_rb(st, bi):
                rb2 = zppool.tile([128, 1024], F32, tag="zp", name=f"rb2_{bi}")
                for p in (0, 1):
                    nc.tensor.matmul(
                        rb2[:, 512 * p : 512 * p + NT],
                        consr["selc"][64 * p : 64 * p + 64, 128 * bi : 128 * bi + 128],
                        st["rstd2"][64 * p : 64 * p + 64, :],
                        start=True, stop=True, tile_position=(64 * p, 0),
                    )
                st["rb2"][bi] = rb2

            def c_ew(st, bi):
                rb2 = st["rb2"][bi]
                zs_2 = st["zs2"][bi]
                nc.vector.tensor_tensor(sv2(zs_2), sv2(zs_2), pv(rb2), ALU.mult)
                h2 = hpool.tile([128, 2 * NT], MMDT, tag="h", name=f"h2_{bi}")
                nc.scalar.activation(h2[:], zs_2[:], gelu, bias=cons["be2c"][:])
                st["h2"][bi] = h2

            def c_ss(st, bi):
                for p in (0, 1):
                    slab_mm(st["zstk"], bi, p, consr["w3sl"],
                            st["h2"][bi][:][:, p * NT : p * NT + NT],
                            first=(bi == 0 and p == 0), last=(bi == WAVE - 1 and p == 1))

            def emit_A(st):
                st["sstk1"] = sspool.tile([128, NT], F32, tag="stk", name="sstk1")
                for k in range(WAVE + 3):
                    if k < WAVE:
                        a_mm(st, k)
                    if 0 <= k - 1 < WAVE:
                        a_ev(st, k - 1)
                    if 0 <= k - 2 < WAVE:
                        a_q(st, k - 2)
                    if 0 <= k - 3 < WAVE:
                        a_ss(st, k - 3)
                st["rstd1"] = nr_rsqrt(st["sstk1"], nc.gpsimd)

            def new_state(w):
                return {"w": w, "x4v": gather(w),
                        "zp1": [None] * WAVE, "zs1": [None] * WAVE, "q1": [None] * WAVE,
                        "rb1": [None] * WAVE, "zp2": [None] * WAVE,
                        "zs2": [None] * WAVE, "q2": [None] * WAVE,
                        "rb2": [None] * WAVE, "h2": [None] * WAVE}

            st = new_state(0)
            emit_A(st)
            for w in range(NW):
                # -- B(w): rstd bcast; LN1 apply; gelu -> h1; L2; evict; sq; SS2 --
                st["sstk2"] = sspool.tile([128, NT], F32, tag="stk", name="sstk2")
                for k in range(WAVE + 3):
                    if k < WAVE:
                        b_rb(st, k)
                    if 0 <= k - 1 < WAVE:
                        b_ew(st, k - 1)
                    if 0 <= k - 2 < WAVE:
                        b_ev(st, k - 2)
                    if 0 <= k - 3 < WAVE:
                        b_ss(st, k - 3)
                st["rstd2"] = nr_rsqrt(st["sstk2"], nc.gpsimd)

                # -- C(w) interleaved with A(w+1) --
                st["zstk"] = sspool.tile([128, NT], F32, tag="stk", name="zstk")
                stn = new_state(w + 1) if w + 1 < NW else None
                if stn is not None:
                    stn["sstk1"] = sspool.tile([128, NT], F32, tag="stk", name="sstk1")
                for k in range(WAVE + 3):
                    if k < WAVE:
                        c_rb(st, k)
                        if stn is not None:
                            a_mm(stn, k)
                    if 0 <= k - 1 < WAVE:
                        c_ew(st, k - 1)
                        if stn is not None:
                            a_ev(stn, k - 1)
                    if 0 <= k - 2 < WAVE:
                        c_ss(st, k - 2)
                        if stn is not None:
                            a_q(stn, k - 2)
                    if 0 <= k - 3 < WAVE and stn is not None:
                        a_ss(stn, k - 3)
                # zeta rows psum->sbuf (bf16), then 4 contiguous-row scatters
                zsc = zcpool.tile([128, NT], MMDT, tag="zsc")
                nc.scalar.activation(zsc[:, :], st["zstk"][:, :], ACTF.Identity)
                for i in range(4):
                    a_, b_ = i // 2, i % 2
                    nc.sync.dma_start(
                        zb[32 * i + HWV * w : 32 * i + HWV * w + HWV, :],
                        zsc[64 * a_ + 32 * b_ : 64 * a_ + 32 * b_ + 32, :],
                    )
                if stn is not None:
                    stn["rstd1"] = nr_rsqrt(stn["sstk1"], nc.gpsimd)
                st = stn

            # ---- phase C: zeta -> Y ----
            tbm = bmpool.tile([128, SEG], F32, tag="dwb")
            nc.scalar.activation(tbm[:], zb[:], ACTF.Tanh, bias=cons["b3h"][:], scale=0.5)
            nc.vector.scalar_tensor_tensor(tbm[:], tbm[:], 1.0, vt[:], ALU.add, ALU.mult)
            ps = bmpool.tile([128, G], F32, tag="ps")
            nc.vector.tensor_reduce(
                ps[:], tbm[:].rearrange("p (g t) -> p g t", t=MSTEPS), mybir.AxisListType.X, ALU.add
            )
            yout = bmpool.tile([128, G], F32, tag="yout")
            nc.vector.tensor_scalar(yout[:], ps[:], 0.5, cons["y0c"][:], ALU.mult, ALU.add)
            nc.sync.dma_start(yo_d.ap().rearrange("(p g) o -> p (g o)", p=128), yout[:])

    nc.compile()
    return nc


_CACHE = {}


def _get_program(G=32):
    if G not in _CACHE:
        _CACHE[G] = build_program(G)
    return _CACHE[G]


def make_in_maps(inputs, G=32):
    BC = 128 * G
    cons = _prep_weights(inputs)
    cons["afull"] = _afull(G)
    dw = np.ascontiguousarray(np.asarray(inputs["dw"], np.float32)[: NCORES * BC])
    tg = np.ascontiguousarray(np.asarray(inputs["t_grid"], np.float32)[: NCORES * BC])
    maps = []
    for c in range(NCORES):
        m = {"dw": dw[c * BC : (c + 1) * BC], "tg": tg[c * BC : (c + 1) * BC]}
        m.update(cons)
        maps.append(m)
    return maps


def kernel(**inputs):
    nc = _get_program()
    in_maps = make_in_maps(inputs)
    res = bass_utils.run_bass_kernel_spmd(nc, in_maps, core_ids=list(range(NCORES)))
    Y = np.concatenate([res.results[c]["yo"] for c in range(NCORES)], axis=0)
    S = np.concatenate([res.results[c]["so"] for c in range(NCORES)], axis=0)
    return Y.reshape(B, 1).astype(np.float32), S.reshape(B, 1).astype(np.float32)
